# revision 23
# baseline (speedup 1.0000x reference)
"""Trainium2 Bass kernel for nn_Attention_18382460027073 (dense transformer attention).

Self-contained: accepts FULL inputs, shards across 8 NeuronCores internally,
returns the FULL output.

Math (faithful to the reference's torch-style .view reshapes):
  X = hidden_states.reshape(8192, 2048)
  The computation decomposes into 64 independent 128-row blocks of X
  (block beta = 16*b + h): the reference's (B, H, S, 3*dh) view maps head h of
  batch b exactly onto rows [128*beta, 128*beta+128) of X, and the attention
  "sequence" axis t of that head is a row-major reinterpretation of the
  (128, 6144) qkv block as (2048, 384).  Each core owns 8 consecutive blocks
  (1024 rows), needs the full weights (replicated), and no collectives.

All inputs are pre-permuted on the host into per-partition-contiguous layouts
so every weight/activation DMA needs only 128 descriptors (descriptor
generation on the Sync queue is the dominant DMA cost otherwise).

Per-core pipeline (matmul operands bf16, fp32 PSUM accumulation):
  1b. q,k projection producing the TRANSPOSED layout (lhsT = w_qkv column
      block, rhs = x^T) -> DRAM scratch in (d, g, r)-major layout
  1a. v projection producing the natural layout (lhsT = x^T chunk,
      rhs = w_qkv v-columns), SBUF-resident
  2+3 fused per block (q^T/k^T reloaded once as two big SBUF tiles):
      scores^T = k^T.T @ q^T over 512-wide j tiles, exp on ScalarE (no max
      subtraction: |scores| = O(6) here), attn^T accumulated with v
      stationary, softmax denominator via an all-ones [128,128] stationary
      matmul (broadcasts the sum to all partitions), reciprocal_approx_fast +
      normalize on VectorE, and the previous block's output projection
      (w_o SBUF-resident) interleaved as dependency-free PE filler.
"""

import sys
import types

import numpy as np
import ml_dtypes


def _install_ntff_hook():
    """antenv.axon_hooks is missing in this image; register the NTFF profile
    hook from trn_agent_boot so run_bass_kernel_spmd(trace=True) works."""
    try:
        import antenv.axon_hooks  # noqa: F401

        return
    except ImportError:
        pass
    hook = None
    try:
        from trn_agent_boot.trn_boot import _ntff_profile_via_ctypes

        hook = _ntff_profile_via_ctypes("/opt/axon/libaxon_pjrt.so")
    except Exception:
        pass
    mod = types.ModuleType("antenv.axon_hooks")
    mod.get_axon_ntff_profile_hook = lambda: hook
    sys.modules["antenv.axon_hooks"] = mod


_install_ntff_hook()

import concourse.bass as bass  # noqa: E402
import concourse.mybir as mybir  # noqa: E402
import concourse.tile as tile  # noqa: E402
from concourse import bacc, bass_utils  # noqa: E402

B, S, HID = 4, 2048, 2048
NH, DH = 16, 128
NQKV = 3 * HID
P = 128
N_CORES = 8
ROWS = (B * S) // N_CORES  # 1024 rows of flattened X per core
NBLK = ROWS // P  # 8 blocks per core
KC = HID // P  # 16 contraction chunks of 128
G = NH  # 16 (g, d) groups per block
SCALE = 1.0 / float(np.sqrt(DH))

F32 = mybir.dt.float32
BF16 = mybir.dt.bfloat16
EXP = mybir.ActivationFunctionType.Exp
BF16_NP = ml_dtypes.bfloat16


def build_nc():
    nc = bacc.Bacc("TRN2", target_bir_lowering=False, debug=False, num_devices=N_CORES)
    # host-pre-permuted inputs (per-partition contiguous):
    #   x_in   [128(p), 16(ck), 1024(r)]   x^T tiled by contraction chunk
    #   wqk_in [32(c=2g+qk), 128(p), 16(ck), 128(d)]
    #   wv_in  [4(nt), 128(p), 16(ck), 4(gg), 128(d)]
    #   wo_in  [128(p), 16(g), 2048(n)]
    x_in = nc.dram_tensor("x_in", [P, KC, ROWS], BF16, kind="ExternalInput").ap()
    wqk_in = nc.dram_tensor("wqk_in", [2 * G, P, KC, DH], BF16, kind="ExternalInput").ap()
    wv_in = nc.dram_tensor("wv_in", [4, P, KC, 4, DH], BF16, kind="ExternalInput").ap()
    wo_in = nc.dram_tensor("wo_in", [P, G, HID], BF16, kind="ExternalInput").ap()
    out = nc.dram_tensor("out", [ROWS, HID], F32, kind="ExternalOutput").ap()
    # scratch in (d, g, r)-major layout: stores are 2KB-contiguous per
    # partition, the reload is two big 128-descriptor DMAs
    qT_dram = nc.dram_tensor("qT_scratch", [2, DH, G, 512], BF16, kind="Internal").ap()
    kT_dram = nc.dram_tensor("kT_scratch", [2, DH, G, 512], BF16, kind="Internal").ap()

    with tile.TileContext(nc) as tc:
        with tc.tile_pool(name="const", bufs=1) as const_pool, tc.tile_pool(
            name="vsb", bufs=1
        ) as v_pool, tc.tile_pool(name="wosb", bufs=1) as wo_pool, tc.tile_pool(
            name="qkh0", bufs=1
        ) as qkh0_pool:
            ones_t = const_pool.tile([P, P], BF16)
            nc.vector.memset(ones_t[:], 1.0)
            v_sb = v_pool.tile([P, NBLK, NH * DH], BF16)  # [128, 8, 2048]
            wo_sb = wo_pool.tile([P, G, HID], BF16)  # [128, 16, 2048]

            # ---------------- Phase 1: projections ----------------
            with tc.tile_pool(name="xsb", bufs=1) as x_pool, tc.tile_pool(
                name="wqk", bufs=2
            ) as wqk_pool, tc.tile_pool(name="wv", bufs=2) as wv_pool, tc.tile_pool(
                name="qkstage", bufs=3
            ) as stage_pool, tc.tile_pool(
                name="ps1b", bufs=2, space="PSUM"
            ) as ps1b, tc.tile_pool(name="ps1a", bufs=1, space="PSUM") as ps1a:
                x_sb = x_pool.tile([P, KC, ROWS], BF16)
                nc.sync.dma_start(x_sb[:], x_in)

                # 1b: q,k projection, transposed layout -> DRAM scratch
                for g in range(G):
                    for qk in range(2):
                        wt = wqk_pool.tile([P, KC, P], BF16, tag="wqk")
                        nc.sync.dma_start(wt[:], wqk_in[2 * g + qk])
                        ps = ps1b.tile([P, ROWS], F32, tag="qk")  # 2 banks
                        for ck in range(KC):
                            for j in range(2):
                                nc.tensor.matmul(
                                    ps[:, j * 512 : (j + 1) * 512],
                                    lhsT=wt[:, ck, :],
                                    rhs=x_sb[:, ck, j * 512 : (j + 1) * 512],
                                    start=(ck == 0),
                                    stop=(ck == KC - 1),
                                )
                        st = stage_pool.tile([P, ROWS], BF16, tag="qkstage")
                        nc.any.tensor_copy(out=st[:], in_=ps[:])
                        dst = qT_dram if qk == 0 else kT_dram
                        for rh in range(2):
                            nc.sync.dma_start(
                                dst[rh, :, g, :], st[:, rh * 512 : (rh + 1) * 512]
                            )
                    if g % 4 == 3:
                        nc.sync.dma_start(
                            wo_sb[:, g - 3 : g + 1, :], wo_in[:, g - 3 : g + 1, :]
                        )

                # reload of q^T/k^T half 0 overlaps the v projection
                qT_h0 = qkh0_pool.tile([P, G, 512], BF16, tag="qT0")
                nc.sync.dma_start(qT_h0[:], qT_dram[0])
                kT_h0 = qkh0_pool.tile([P, G, 512], BF16, tag="kT0")
                nc.sync.dma_start(kT_h0[:], kT_dram[0])

                # 1a: v projection, natural layout, stays in SBUF
                for nt in range(4):
                    wv = wv_pool.tile([P, KC, 4, DH], BF16, tag="wv")
                    nc.sync.dma_start(wv[:], wv_in[nt])
                    for ihalf in range(2):
                        psv = [
                            ps1a.tile([P, 512], F32, tag=f"v{il}", name=f"psv{il}")
                            for il in range(4)
                        ]
                        for ck in range(KC):
                            for il in range(4):
                                i = ihalf * 4 + il
                                nc.tensor.matmul(
                                    psv[il][:],
                                    lhsT=x_sb[:, ck, i * P : (i + 1) * P],
                                    rhs=wv[:, ck, :, :],
                                    start=(ck == 0),
                                    stop=(ck == KC - 1),
                                )
                        for il in range(4):
                            i = ihalf * 4 + il
                            nc.any.tensor_copy(
                                out=v_sb[:, i, nt * 512 : (nt + 1) * 512],
                                in_=psv[il][:],
                            )

            # ---------------- Phase 2+3 fused: attention + o-proj ----------------
            with tc.tile_pool(name="qkh1", bufs=1) as qkh1_pool, tc.tile_pool(
                name="attnT", bufs=2
            ) as at_pool, tc.tile_pool(name="probsT", bufs=8) as pt_pool, tc.tile_pool(
                name="bcast", bufs=3
            ) as bc_pool, tc.tile_pool(name="ostage", bufs=3) as out_pool, tc.tile_pool(
                name="ps2a", bufs=2, space="PSUM"
            ) as ps2a, tc.tile_pool(name="ps2e", bufs=1, space="PSUM") as ps2e, tc.tile_pool(
                name="ps2s", bufs=3, space="PSUM"
            ) as ps2s, tc.tile_pool(name="ps2o", bufs=2, space="PSUM") as ps2o:
                qT_h1 = qkh1_pool.tile([P, G, 512], BF16, tag="qT1")
                nc.sync.dma_start(qT_h1[:], qT_dram[1])
                kT_h1 = qkh1_pool.tile([P, G, 512], BF16, tag="kT1")
                nc.sync.dma_start(kT_h1[:], kT_dram[1])

                attn_tiles = {}

                def emit_oproj_quarter(i, nq):
                    ps_o = ps2o.tile([P, 512], F32, tag="o", name="ps_o")
                    for g in range(G):
                        nc.tensor.matmul(
                            ps_o[:],
                            lhsT=attn_tiles[i][:, g * P : (g + 1) * P],
                            rhs=wo_sb[:, g, nq * 512 : (nq + 1) * 512],
                            start=(g == 0),
                            stop=(g == G - 1),
                        )
                    st = out_pool.tile([P, 512], F32, tag="ostage")
                    nc.vector.tensor_copy(out=st[:], in_=ps_o[:])
                    nc.sync.dma_start(
                        out[i * P : (i + 1) * P, nq * 512 : (nq + 1) * 512],
                        st[:],
                    )

                for i in range(NBLK):
                    qT_c = qT_h0 if i < 4 else qT_h1
                    kT_c = kT_h0 if i < 4 else kT_h1
                    rsl = slice((i % 4) * P, (i % 4 + 1) * P)
                    attn_blk = at_pool.tile([P, S], BF16, tag="attnT", name="attn_blk")
                    attn_tiles[i] = attn_blk
                    for jt in range(4):  # 512-wide j tiles
                        ps_a = ps2a.tile([P, 512], F32, tag="a", name="ps_a")
                        ps_e = ps2e.tile([P, 512], F32, tag="e", name="ps_e")
                        for gk in range(G):
                            ps_s = ps2s.tile([P, 512], F32, tag="s", name="ps_s")
                            nc.tensor.matmul(
                                ps_s[:],
                                lhsT=kT_c[:, gk, rsl],
                                rhs=qT_c[:, 4 * jt : 4 * jt + 4, rsl],
                                start=True,
                                stop=True,
                            )
                            pb = pt_pool.tile([P, 512], BF16, tag="pT")
                            nc.scalar.activation(pb[:], ps_s[:], EXP, scale=SCALE)
                            nc.tensor.matmul(
                                ps_a[:],
                                lhsT=v_sb[:, i, gk * DH : (gk + 1) * DH],
                                rhs=pb[:],
                                start=(gk == 0),
                                stop=(gk == G - 1),
                            )
                            nc.tensor.matmul(
                                ps_e[:],
                                lhsT=ones_t[:, :],
                                rhs=pb[:],
                                start=(gk == 0),
                                stop=(gk == G - 1),
                            )
                        # sumexp is broadcast across all 128 partitions of ps_e
                        bc = bc_pool.tile([P, 512], F32, tag="bcast")
                        nc.vector.reciprocal_approx_fast(out=bc[:], in_=ps_e[:])
                        nc.vector.tensor_mul(
                            out=attn_blk[:, jt * 512 : (jt + 1) * 512],
                            in0=ps_a[:],
                            in1=bc[:],
                        )
                        # o-proj quarter of the PREVIOUS block as dependency-free
                        # PE filler for the exp/normalize latency
                        if i > 0:
                            emit_oproj_quarter(i - 1, jt)
                for nq in range(4):
                    emit_oproj_quarter(NBLK - 1, nq)

    nc.compile()
    return nc


_NC_CACHE = None


def _get_nc():
    global _NC_CACHE
    if _NC_CACHE is None:
        _NC_CACHE = build_nc()
    return _NC_CACHE


def make_in_maps(hidden_states, w_qkv, w_o):
    X = np.asarray(hidden_states, dtype=np.float32).reshape(B * S, HID)
    shards = X.reshape(N_CORES, ROWS, HID)
    wq = np.asarray(w_qkv, dtype=np.float32).astype(BF16_NP)
    wo = np.asarray(w_o, dtype=np.float32).astype(BF16_NP)

    # wqk_in [32(2g+qk), 128(p), 16(ck), 128(d)]
    W5 = wq.reshape(KC, P, G, 3, DH)  # (ck, p, g, e3, d)
    wqk = np.ascontiguousarray(
        W5[:, :, :, :2, :].transpose(2, 3, 1, 0, 4).reshape(2 * G, P, KC, DH)
    )
    # wv_in [4(nt), 128(p), 16(ck), 4(gg), 128(d)]
    wv = np.ascontiguousarray(
        W5[:, :, :, 2, :]  # (ck, p, g, d)
        .transpose(2, 1, 0, 3)  # (g, p, ck, d)
        .reshape(4, 4, P, KC, DH)  # (nt, gg, p, ck, d)
        .transpose(0, 2, 3, 1, 4)  # (nt, p, ck, gg, d)
    )
    # wo_in [128(p), 16(g), 2048(n)]
    wo_p = np.ascontiguousarray(wo.reshape(G, P, HID).transpose(1, 0, 2))

    in_maps = []
    for c in range(N_CORES):
        # x_in [128(p), 16(ck), 1024(r)]
        xT = shards[c].T.astype(BF16_NP)  # (2048, 1024)
        x_p = np.ascontiguousarray(xT.reshape(KC, P, ROWS).transpose(1, 0, 2))
        in_maps.append({"x_in": x_p, "wqk_in": wqk, "wv_in": wv, "wo_in": wo_p})
    return in_maps


def assemble_output(results):
    outs = [results[c]["out"] for c in range(N_CORES)]
    return np.concatenate(outs, axis=0).reshape(B, S, HID).astype(np.float32)


def kernel(hidden_states, w_qkv, w_o):
    nc = _get_nc()
    in_maps = make_in_maps(hidden_states, w_qkv, w_o)
    res = bass_utils.run_bass_kernel_spmd(nc, in_maps, core_ids=list(range(N_CORES)))
    return assemble_output(res.results)


# revision 24
# speedup vs baseline: 1.0020x; 1.0020x over previous
"""Trainium2 Bass kernel for nn_Attention_18382460027073 (dense transformer attention).

Self-contained: accepts FULL inputs, shards across 8 NeuronCores internally,
returns the FULL output.

Math (faithful to the reference's torch-style .view reshapes):
  X = hidden_states.reshape(8192, 2048)
  The computation decomposes into 64 independent 128-row blocks of X
  (block beta = 16*b + h): the reference's (B, H, S, 3*dh) view maps head h of
  batch b exactly onto rows [128*beta, 128*beta+128) of X, and the attention
  "sequence" axis t of that head is a row-major reinterpretation of the
  (128, 6144) qkv block as (2048, 384).  Each core owns 8 consecutive blocks
  (1024 rows), needs the full weights (replicated), and no collectives.

All inputs are pre-permuted on the host into per-partition-contiguous layouts
so every weight/activation DMA needs only 128 descriptors (descriptor
generation on the Sync queue is the dominant DMA cost otherwise).

Per-core pipeline (matmul operands bf16, fp32 PSUM accumulation):
  1b. q,k projection producing the TRANSPOSED layout (lhsT = w_qkv column
      block, rhs = x^T) -> DRAM scratch in (d, g, r)-major layout
  1a. v projection producing the natural layout (lhsT = x^T chunk,
      rhs = w_qkv v-columns), SBUF-resident
  2+3 fused per block (q^T/k^T reloaded once as two big SBUF tiles):
      scores^T = k^T.T @ q^T over 512-wide j tiles, exp on ScalarE (no max
      subtraction: |scores| = O(6) here), attn^T accumulated with v
      stationary, softmax denominator via an all-ones [128,128] stationary
      matmul (broadcasts the sum to all partitions), reciprocal_approx_fast +
      normalize on VectorE, and the previous block's output projection
      (w_o SBUF-resident) interleaved as dependency-free PE filler.
"""

import sys
import types

import numpy as np
import ml_dtypes


def _install_ntff_hook():
    """antenv.axon_hooks is missing in this image; register the NTFF profile
    hook from trn_agent_boot so run_bass_kernel_spmd(trace=True) works."""
    try:
        import antenv.axon_hooks  # noqa: F401

        return
    except ImportError:
        pass
    hook = None
    try:
        from trn_agent_boot.trn_boot import _ntff_profile_via_ctypes

        hook = _ntff_profile_via_ctypes("/opt/axon/libaxon_pjrt.so")
    except Exception:
        pass
    mod = types.ModuleType("antenv.axon_hooks")
    mod.get_axon_ntff_profile_hook = lambda: hook
    sys.modules["antenv.axon_hooks"] = mod


_install_ntff_hook()

import concourse.bass as bass  # noqa: E402
import concourse.mybir as mybir  # noqa: E402
import concourse.tile as tile  # noqa: E402
from concourse import bacc, bass_utils  # noqa: E402

B, S, HID = 4, 2048, 2048
NH, DH = 16, 128
NQKV = 3 * HID
P = 128
N_CORES = 8
ROWS = (B * S) // N_CORES  # 1024 rows of flattened X per core
NBLK = ROWS // P  # 8 blocks per core
KC = HID // P  # 16 contraction chunks of 128
G = NH  # 16 (g, d) groups per block
SCALE = 1.0 / float(np.sqrt(DH))

F32 = mybir.dt.float32
BF16 = mybir.dt.bfloat16
EXP = mybir.ActivationFunctionType.Exp
BF16_NP = ml_dtypes.bfloat16


def build_nc():
    nc = bacc.Bacc("TRN2", target_bir_lowering=False, debug=False, num_devices=N_CORES)
    # host-pre-permuted inputs (per-partition contiguous):
    #   x_in   [128(p), 16(ck), 1024(r)]   x^T tiled by contraction chunk
    #   wqk_in [32(c=2g+qk), 128(p), 16(ck), 128(d)]
    #   wv_in  [4(nt), 128(p), 16(ck), 4(gg), 128(d)]
    #   wo_in  [128(p), 16(g), 2048(n)]
    x_in = nc.dram_tensor("x_in", [P, KC, ROWS], BF16, kind="ExternalInput").ap()
    wqk_in = nc.dram_tensor("wqk_in", [2 * G, P, KC, DH], BF16, kind="ExternalInput").ap()
    wv_in = nc.dram_tensor("wv_in", [4, P, KC, 4, DH], BF16, kind="ExternalInput").ap()
    wo_in = nc.dram_tensor("wo_in", [P, G, HID], BF16, kind="ExternalInput").ap()
    out = nc.dram_tensor("out", [ROWS, HID], F32, kind="ExternalOutput").ap()
    # scratch in (d, g, r)-major layout: stores are 2KB-contiguous per
    # partition, the reload is two big 128-descriptor DMAs
    qT_dram = nc.dram_tensor("qT_scratch", [2, DH, G, 512], BF16, kind="Internal").ap()
    kT_dram = nc.dram_tensor("kT_scratch", [2, DH, G, 512], BF16, kind="Internal").ap()

    with tile.TileContext(nc) as tc:
        with tc.tile_pool(name="const", bufs=1) as const_pool, tc.tile_pool(
            name="vsb", bufs=1
        ) as v_pool, tc.tile_pool(name="wosb", bufs=1) as wo_pool, tc.tile_pool(
            name="qkh0", bufs=1
        ) as qkh0_pool:
            ones_t = const_pool.tile([P, P], BF16)
            nc.vector.memset(ones_t[:], 1.0)
            v_sb = v_pool.tile([P, NBLK, NH * DH], BF16)  # [128, 8, 2048]
            wo_sb = wo_pool.tile([P, G, HID], BF16)  # [128, 16, 2048]

            # ---------------- Phase 1: projections ----------------
            with tc.tile_pool(name="xsb", bufs=1) as x_pool, tc.tile_pool(
                name="wqk", bufs=2
            ) as wqk_pool, tc.tile_pool(name="wv", bufs=2) as wv_pool, tc.tile_pool(
                name="qkstage", bufs=3
            ) as stage_pool, tc.tile_pool(
                name="ps1b", bufs=2, space="PSUM"
            ) as ps1b, tc.tile_pool(name="ps1a", bufs=1, space="PSUM") as ps1a:
                x_sb = x_pool.tile([P, KC, ROWS], BF16)
                nc.sync.dma_start(x_sb[:], x_in)

                # 1b: q,k projection, transposed layout -> DRAM scratch
                for g in range(G):
                    for qk in range(2):
                        wt = wqk_pool.tile([P, KC, P], BF16, tag="wqk")
                        nc.sync.dma_start(wt[:], wqk_in[2 * g + qk])
                        ps = ps1b.tile([P, ROWS], F32, tag="qk")  # 2 banks
                        for ck in range(KC):
                            for j in range(2):
                                nc.tensor.matmul(
                                    ps[:, j * 512 : (j + 1) * 512],
                                    lhsT=wt[:, ck, :],
                                    rhs=x_sb[:, ck, j * 512 : (j + 1) * 512],
                                    start=(ck == 0),
                                    stop=(ck == KC - 1),
                                )
                        st = stage_pool.tile([P, ROWS], BF16, tag="qkstage")
                        nc.any.tensor_copy(out=st[:], in_=ps[:])
                        dst = qT_dram if qk == 0 else kT_dram
                        for rh in range(2):
                            nc.sync.dma_start(
                                dst[rh, :, g, :], st[:, rh * 512 : (rh + 1) * 512]
                            )
                    if g % 4 == 3:
                        nc.sync.dma_start(
                            wo_sb[:, g - 3 : g + 1, :], wo_in[:, g - 3 : g + 1, :]
                        )

                # reload of q^T/k^T half 0 overlaps the v projection
                qT_h0 = qkh0_pool.tile([P, G, 512], BF16, tag="qT0")
                nc.sync.dma_start(qT_h0[:], qT_dram[0])
                kT_h0 = qkh0_pool.tile([P, G, 512], BF16, tag="kT0")
                nc.sync.dma_start(kT_h0[:], kT_dram[0])

                # 1a: v projection, natural layout, stays in SBUF
                for nt in range(4):
                    wv = wv_pool.tile([P, KC, 4, DH], BF16, tag="wv")
                    nc.sync.dma_start(wv[:], wv_in[nt])
                    for ihalf in range(2):
                        psv = [
                            ps1a.tile([P, 512], F32, tag=f"v{il}", name=f"psv{il}")
                            for il in range(4)
                        ]
                        for ck in range(KC):
                            for il in range(4):
                                i = ihalf * 4 + il
                                nc.tensor.matmul(
                                    psv[il][:],
                                    lhsT=x_sb[:, ck, i * P : (i + 1) * P],
                                    rhs=wv[:, ck, :, :],
                                    start=(ck == 0),
                                    stop=(ck == KC - 1),
                                )
                        for il in range(4):
                            i = ihalf * 4 + il
                            nc.any.tensor_copy(
                                out=v_sb[:, i, nt * 512 : (nt + 1) * 512],
                                in_=psv[il][:],
                            )

            # ---------------- Phase 2+3 fused: attention + o-proj ----------------
            with tc.tile_pool(name="qkh1", bufs=1) as qkh1_pool, tc.tile_pool(
                name="attnT", bufs=2
            ) as at_pool, tc.tile_pool(name="probsT", bufs=6) as pt_pool, tc.tile_pool(
                name="bcast", bufs=2
            ) as bc_pool, tc.tile_pool(name="ostage", bufs=2) as out_pool, tc.tile_pool(
                name="ps2a", bufs=2, space="PSUM"
            ) as ps2a, tc.tile_pool(name="ps2e", bufs=1, space="PSUM") as ps2e, tc.tile_pool(
                name="ps2s", bufs=3, space="PSUM"
            ) as ps2s, tc.tile_pool(name="ps2o", bufs=2, space="PSUM") as ps2o:
                qT_h1 = qkh1_pool.tile([P, G, 512], BF16, tag="qT1")
                nc.sync.dma_start(qT_h1[:], qT_dram[1])
                kT_h1 = qkh1_pool.tile([P, G, 512], BF16, tag="kT1")
                nc.sync.dma_start(kT_h1[:], kT_dram[1])

                attn_tiles = {}

                def emit_oproj_quarter(i, nq):
                    ps_o = ps2o.tile([P, 512], F32, tag="o", name="ps_o")
                    for g in range(G):
                        nc.tensor.matmul(
                            ps_o[:],
                            lhsT=attn_tiles[i][:, g * P : (g + 1) * P],
                            rhs=wo_sb[:, g, nq * 512 : (nq + 1) * 512],
                            start=(g == 0),
                            stop=(g == G - 1),
                        )
                    st = out_pool.tile([P, 512], F32, tag="ostage")
                    nc.vector.tensor_copy(out=st[:], in_=ps_o[:])
                    nc.sync.dma_start(
                        out[i * P : (i + 1) * P, nq * 512 : (nq + 1) * 512],
                        st[:],
                    )

                for i in range(NBLK):
                    qT_c = qT_h0 if i < 4 else qT_h1
                    kT_c = kT_h0 if i < 4 else kT_h1
                    rsl = slice((i % 4) * P, (i % 4 + 1) * P)
                    attn_blk = at_pool.tile([P, S], BF16, tag="attnT", name="attn_blk")
                    attn_tiles[i] = attn_blk
                    for jt in range(4):  # 512-wide j tiles
                        ps_a = ps2a.tile([P, 512], F32, tag="a", name="ps_a")
                        ps_e = ps2e.tile([P, 512], F32, tag="e", name="ps_e")
                        for gk in range(G):
                            ps_s = ps2s.tile([P, 512], F32, tag="s", name="ps_s")
                            nc.tensor.matmul(
                                ps_s[:],
                                lhsT=kT_c[:, gk, rsl],
                                rhs=qT_c[:, 4 * jt : 4 * jt + 4, rsl],
                                start=True,
                                stop=True,
                            )
                            pb = pt_pool.tile([P, 512], BF16, tag="pT")
                            nc.scalar.activation(pb[:], ps_s[:], EXP, scale=SCALE)
                            nc.tensor.matmul(
                                ps_a[:],
                                lhsT=v_sb[:, i, gk * DH : (gk + 1) * DH],
                                rhs=pb[:],
                                start=(gk == 0),
                                stop=(gk == G - 1),
                            )
                            nc.tensor.matmul(
                                ps_e[:],
                                lhsT=ones_t[:, :],
                                rhs=pb[:],
                                start=(gk == 0),
                                stop=(gk == G - 1),
                            )
                        # sumexp is broadcast across all 128 partitions of ps_e
                        bc = bc_pool.tile([P, 512], F32, tag="bcast")
                        nc.vector.reciprocal_approx_fast(out=bc[:], in_=ps_e[:])
                        nc.vector.tensor_mul(
                            out=attn_blk[:, jt * 512 : (jt + 1) * 512],
                            in0=ps_a[:],
                            in1=bc[:],
                        )
                        # o-proj quarter of the PREVIOUS block as dependency-free
                        # PE filler for the exp/normalize latency
                        if i > 0:
                            emit_oproj_quarter(i - 1, jt)
                for nq in range(4):
                    emit_oproj_quarter(NBLK - 1, nq)

    nc.compile()
    return nc


_NC_CACHE = None


def _get_nc():
    global _NC_CACHE
    if _NC_CACHE is None:
        _NC_CACHE = build_nc()
    return _NC_CACHE


def make_in_maps(hidden_states, w_qkv, w_o):
    X = np.asarray(hidden_states, dtype=np.float32).reshape(B * S, HID)
    shards = X.reshape(N_CORES, ROWS, HID)
    wq = np.asarray(w_qkv, dtype=np.float32).astype(BF16_NP)
    wo = np.asarray(w_o, dtype=np.float32).astype(BF16_NP)

    # wqk_in [32(2g+qk), 128(p), 16(ck), 128(d)]
    W5 = wq.reshape(KC, P, G, 3, DH)  # (ck, p, g, e3, d)
    wqk = np.ascontiguousarray(
        W5[:, :, :, :2, :].transpose(2, 3, 1, 0, 4).reshape(2 * G, P, KC, DH)
    )
    # wv_in [4(nt), 128(p), 16(ck), 4(gg), 128(d)]
    wv = np.ascontiguousarray(
        W5[:, :, :, 2, :]  # (ck, p, g, d)
        .transpose(2, 1, 0, 3)  # (g, p, ck, d)
        .reshape(4, 4, P, KC, DH)  # (nt, gg, p, ck, d)
        .transpose(0, 2, 3, 1, 4)  # (nt, p, ck, gg, d)
    )
    # wo_in [128(p), 16(g), 2048(n)]
    wo_p = np.ascontiguousarray(wo.reshape(G, P, HID).transpose(1, 0, 2))

    in_maps = []
    for c in range(N_CORES):
        # x_in [128(p), 16(ck), 1024(r)]
        xT = shards[c].T.astype(BF16_NP)  # (2048, 1024)
        x_p = np.ascontiguousarray(xT.reshape(KC, P, ROWS).transpose(1, 0, 2))
        in_maps.append({"x_in": x_p, "wqk_in": wqk, "wv_in": wv, "wo_in": wo_p})
    return in_maps


def assemble_output(results):
    outs = [results[c]["out"] for c in range(N_CORES)]
    return np.concatenate(outs, axis=0).reshape(B, S, HID).astype(np.float32)


def kernel(hidden_states, w_qkv, w_o):
    nc = _get_nc()
    in_maps = make_in_maps(hidden_states, w_qkv, w_o)
    res = bass_utils.run_bass_kernel_spmd(nc, in_maps, core_ids=list(range(N_CORES)))
    return assemble_output(res.results)
